# revision 3
# baseline (speedup 1.0000x reference)
"""Trainium2 Bass kernel for nn_CooccurrenceMatrix.

Math (per batch b):
  onehot S[(v,p), w] = [nodes[b,w,p]==v] * mask[b,w,p]        (one-hot over ids)
  A_T = blockdiag(K) @ S                                      ([(v,q), w])
  cooc_raw[w,x] = sum_{(v,q)} A_T[(v,q),w] * S[(v,q),x]
  dc[w] = #singleton-group members in walk w  (groups of size 1 get their
          self-pair K[p,p]=1 removed from the diagonal)
  out[w,x] = (cooc_raw - diag(dc)) * r[w]*r[x],  r = 1/max(lens,1)

Implementation: fully data-parallel over batches; 8 batches per NeuronCore.
All matmuls in bf16 (S, A exact in bf16; K rounded -> ~0.2% rel err).
The contraction dim (v,q)=400 is split into chunks of 120/120/120/40
(6 id-blocks of 20 positions per chunk) so PSUM accumulates 4 matmuls
per output row-chunk. r[x] is pre-folded into the cooc rhs; r[w] is a
per-partition scale on the PSUM->SBUF output copy; the singleton diagonal
correction is injected into PSUM with one small matmul per row-chunk.
"""

import numpy as np

B, W, L, V = 64, 512, 20, 20
SIGMA = 2.0
NCORES = 8
BPC = B // NCORES            # batches per core
WC = W // 128                # 4 row chunks of 128 walks
CHUNK_ROWS = (120, 120, 120, 40)   # (v,p) contraction chunk heights
CHUNK_VS = (6, 6, 6, 2)            # id-blocks per chunk
PF = BPC * WC * L            # free size of the per-core node tile (8*4*20=640)
NF = BPC * 512               # all-batch walk columns (4096)

_CACHE = {}


def _consts():
    import ml_dtypes

    bf16 = ml_dtypes.bfloat16
    pos = np.arange(L, dtype=np.float64)
    K = np.exp(-((pos[:, None] - pos[None, :]) ** 2) / (SIGMA ** 2))
    Kb = K.astype(bf16)

    bdkA = np.zeros((120, 120), dtype=bf16)       # blockdiag(K) for 6 id-blocks
    for vb in range(6):
        bdkA[vb * L:(vb + 1) * L, vb * L:(vb + 1) * L] = Kb
    bdkB = bdkA[:40, :40].copy()                  # 2-block version

    onesblk = np.zeros((120, 6), dtype=np.float32)  # [p-row, id-block] membership
    for vb in range(6):
        onesblk[vb * L:(vb + 1) * L, vb] = 1.0
    onesblk2 = onesblk[:40, :2].copy()

    E6 = np.zeros((6, 120), dtype=bf16)           # expand per-block -> per-(block,p)
    for vb in range(6):
        E6[vb, vb * L:(vb + 1) * L] = 1.0

    ident = np.eye(128, dtype=bf16)
    negident = (-np.eye(128)).astype(bf16)

    vvec1 = np.zeros((128, 4), dtype=np.float32)  # compare value (id+1) per chunk
    for c in range(3):
        for r in range(120):
            vvec1[r, c] = 6 * c + r // L + 1
    for r in range(40):
        vvec1[r, 3] = 18 + r // L + 1

    return dict(bdkA=bdkA, bdkB=bdkB, onesblk=onesblk, onesblk2=onesblk2,
                E6=E6, ident=ident, negident=negident, vvec1=vvec1)


def _build():
    import concourse.bacc as bacc
    import concourse.mybir as mybir
    import concourse.tile as tile

    dt = mybir.dt
    f32, bf16, i32 = dt.float32, dt.bfloat16, dt.int32
    Alu = mybir.AluOpType
    X = mybir.AxisListType.X

    nc = bacc.Bacc("TRN2", target_bir_lowering=False, debug=False,
                   enable_asserts=False, num_devices=NCORES)

    nodes_d = nc.dram_tensor("nodes", [BPC, W, L], i32, kind="ExternalInput").ap()
    masks_d = nc.dram_tensor("masks", [BPC, W, L], i32, kind="ExternalInput").ap()
    bdkA_d = nc.dram_tensor("bdkA", [120, 120], bf16, kind="ExternalInput").ap()
    bdkB_d = nc.dram_tensor("bdkB", [40, 40], bf16, kind="ExternalInput").ap()
    onesblk_d = nc.dram_tensor("onesblk", [120, 6], f32, kind="ExternalInput").ap()
    onesblk2_d = nc.dram_tensor("onesblk2", [40, 2], f32, kind="ExternalInput").ap()
    E6_d = nc.dram_tensor("E6", [6, 120], bf16, kind="ExternalInput").ap()
    ident_d = nc.dram_tensor("ident", [128, 128], bf16, kind="ExternalInput").ap()
    negident_d = nc.dram_tensor("negident", [128, 128], bf16, kind="ExternalInput").ap()
    vvec1_d = nc.dram_tensor("vvec1", [128, 4], f32, kind="ExternalInput").ap()
    out_d = nc.dram_tensor("out", [BPC, W, W], f32, kind="ExternalOutput").ap()

    with tile.TileContext(nc) as tc:
        with (
            tc.tile_pool(name="const", bufs=1) as cpool,
            tc.tile_pool(name="inp", bufs=1) as ipool,
            tc.tile_pool(name="work", bufs=2) as wpool,
            tc.tile_pool(name="outp", bufs=3) as opool,
            tc.tile_pool(name="ptr", bufs=2, space="PSUM") as ptr,
            tc.tile_pool(name="pvec", bufs=1, space="PSUM") as pvec,
            tc.tile_pool(name="pa", bufs=2, space="PSUM") as pa,
            tc.tile_pool(name="pcooc", bufs=2, space="PSUM") as pcooc,
            tc.tile_pool(name="psmall", bufs=1, space="PSUM") as psmall,
        ):
            # ---- constants to SBUF ----
            bdkA = cpool.tile([120, 120], bf16)
            nc.sync.dma_start(bdkA[:], bdkA_d)
            bdkB = cpool.tile([40, 40], bf16)
            nc.sync.dma_start(bdkB[:], bdkB_d)
            onesblk = cpool.tile([120, 6], f32)
            nc.sync.dma_start(onesblk[:], onesblk_d)
            onesblk2 = cpool.tile([40, 2], f32)
            nc.sync.dma_start(onesblk2[:], onesblk2_d)
            E6 = cpool.tile([6, 120], bf16)
            nc.sync.dma_start(E6[:], E6_d)
            ident = cpool.tile([128, 128], bf16)
            nc.sync.dma_start(ident[:], ident_d)
            negident = cpool.tile([128, 128], bf16)
            nc.sync.dma_start(negident[:], negident_d)
            vvec1 = cpool.tile([128, 4], f32)
            nc.sync.dma_start(vvec1[:], vvec1_d)

            # ---- all-batch input load + prep ----
            nd = ipool.tile([128, PF], i32)      # [w_lane, (b, wc, p)]
            nc.sync.dma_start(
                nd[:].rearrange("wl (b wc p) -> wl b wc p", b=BPC, wc=WC),
                nodes_d.rearrange("b (wc wl) p -> wl b wc p", wl=128))
            mk = ipool.tile([128, PF], i32)
            nc.sync.dma_start(
                mk[:].rearrange("wl (b wc p) -> wl b wc p", b=BPC, wc=WC),
                masks_d.rearrange("b (wc wl) p -> wl b wc p", wl=128))

            # ne1 = (nodes+1)*mask  (0 = masked, else id+1)
            ne1 = ipool.tile([128, PF], bf16)
            nc.vector.scalar_tensor_tensor(
                ne1[:], nd[:], 1.0, mk[:], op0=Alu.add, op1=Alu.mult)

            # lens -> r = 1/max(lens,1), per (b, wc) column
            lens = ipool.tile([128, BPC * WC], f32)
            nc.vector.reduce_sum(
                lens[:], mk[:].rearrange("wl (bw p) -> wl bw p", p=L), axis=X)
            r_wm = ipool.tile([128, BPC * WC], f32)
            nc.vector.tensor_scalar_max(lens[:], lens[:], 1.0)
            nc.vector.reciprocal(r_wm[:], lens[:])
            r_bf = ipool.tile([128, BPC * WC], bf16)
            nc.vector.tensor_copy(r_bf[:], r_wm[:])

            ne1v = ne1[:].rearrange("wl (b wc p) -> wl b wc p", b=BPC, wc=WC)

            # ---- phase 1: transposes for all batches ----
            # netall: [20 pos, (b, 512 walks)]; rx_all: [1, (b, 512 walks)]
            netall = ipool.tile([120, NF], bf16)
            rx_all = ipool.tile([1, NF], bf16)
            for b in range(BPC):
                psum_tr = ptr.tile([20, 512], bf16, tag="ptr")
                for wc in range(WC):
                    nc.tensor.transpose(
                        psum_tr[:, wc * 128:(wc + 1) * 128], ne1v[:, b, wc, :],
                        ident[:])
                nc.scalar.copy(netall[0:20, b * 512:(b + 1) * 512], psum_tr[:])
                psum_row = pvec.tile([1, 512], bf16, tag="prow")
                for m in range(WC):
                    nc.tensor.transpose(
                        psum_row[:, m * 128:(m + 1) * 128],
                        r_bf[:, b * WC + m:b * WC + m + 1], ident[:])
                nc.scalar.copy(rx_all[:, b * 512:(b + 1) * 512], psum_row[:])

            # ---- phase 2: replicate across partitions ----
            # DMA is the only engine free of the 32-aligned partition-base
            # rule, so position-block replication goes through it.
            nc.sync.dma_start(netall[20:40, :], netall[0:20, :])
            nc.sync.dma_start(netall[40:80, :], netall[0:40, :])
            nc.sync.dma_start(netall[80:120, :], netall[0:40, :])
            rrep = ipool.tile([128, NF], bf16)
            nc.gpsimd.partition_broadcast(rrep[:], rx_all[:], channels=128)

            # ---- phase 3: per-batch compute ----
            for b in range(BPC):
                net = netall[:, b * 512:(b + 1) * 512]
                rrep_b = rrep[:, b * 512:(b + 1) * 512]

                psmall_t = psmall.tile([128, 512], f32, tag="small")
                s_raw, s_scl, a_sb = [], [], []
                for c in range(4):
                    R = CHUNK_ROWS[c]
                    nv = CHUNK_VS[c]
                    sr = wpool.tile([120, 512], bf16, tag=f"sraw{c}")
                    acc = wpool.tile([120, 1], f32, tag=f"acc{c}")
                    nc.vector.tensor_scalar(
                        sr[0:R, :], net[0:R, :], vvec1[0:R, c:c + 1], 1.0,
                        op0=Alu.is_equal, op1=Alu.mult, accum_out=acc[0:R, :])
                    ss = wpool.tile([120, 512], bf16, tag=f"sscl{c}")
                    nc.vector.scalar_tensor_tensor(
                        ss[0:R, :], net[0:R, :], vvec1[0:R, c:c + 1],
                        rrep_b[0:R, :], op0=Alu.is_equal, op1=Alu.mult)
                    # A_T chunk
                    pa_t = pa.tile([120, 512], f32, tag="pa")
                    lhs = bdkA[:] if R == 120 else bdkB[:]
                    nc.tensor.matmul(pa_t[0:R, :], lhs, sr[0:R, :],
                                     start=True, stop=True)
                    asb = wpool.tile([120, 512], bf16, tag=f"asb{c}")
                    nc.scalar.copy(asb[0:R, :], pa_t[0:R, :])
                    # group sizes cnt_v (fp32, exact)
                    ob = onesblk[:] if R == 120 else onesblk2[:]
                    nc.tensor.matmul(psmall_t[0:nv, 80 + c:81 + c], ob,
                                     acc[0:R, :], start=True, stop=True)
                    s_raw.append(sr); s_scl.append(ss); a_sb.append(asb)

                # singleton indicator per id, expanded to (id, p) rows
                sing = wpool.tile([6, 4], bf16, tag="sing")
                nc.vector.tensor_scalar(
                    sing[:], psmall_t[0:6, 80:84], 1.0, None, op0=Alu.is_equal)
                for c in range(4):
                    R = CHUNK_ROWS[c]
                    nv = CHUNK_VS[c]
                    nc.tensor.matmul(
                        psmall_t[0:R, 88 + c:89 + c], E6[0:nv, 0:R],
                        sing[0:nv, c:c + 1], start=True, stop=True)
                sv = wpool.tile([120, 4], bf16, tag="sv")
                nc.scalar.copy(sv[:], psmall_t[0:120, 88:92])

                # ---- cooc matmuls + diag-correction column ----
                pc = []
                for m in range(WC):
                    pc_t = pcooc.tile([128, 512], f32, tag="pcooc")
                    for c in range(4):
                        R = CHUNK_ROWS[c]
                        nc.tensor.matmul(
                            pc_t[:], a_sb[c][0:R, m * 128:(m + 1) * 128],
                            s_scl[c][0:R, :], start=(c == 0), stop=False)
                        nc.tensor.matmul(
                            psmall_t[:, 84 + m:85 + m],
                            s_raw[c][0:R, m * 128:(m + 1) * 128],
                            sv[0:R, c:c + 1], start=(c == 0), stop=(c == 3))
                    pc.append(pc_t)

                dcv = wpool.tile([128, 4], f32, tag="dcv")
                nc.scalar.copy(dcv[:], psmall_t[:, 84:88])

                # ---- diagonal inject, normalize, store ----
                for m in range(WC):
                    dmat = wpool.tile([128, 128], bf16, tag="dmat")
                    nc.vector.tensor_scalar(
                        dmat[:], negident[:], dcv[:, m:m + 1],
                        r_wm[:, b * WC + m:b * WC + m + 1],
                        op0=Alu.mult, op1=Alu.mult)
                    nc.tensor.matmul(pc[m][:, m * 128:(m + 1) * 128], dmat[:],
                                     ident[:], start=False, stop=True)
                    osb = opool.tile([128, 512], f32, tag="osb")
                    nc.scalar.activation(
                        osb[:], pc[m][:],
                        mybir.ActivationFunctionType.Copy,
                        scale=r_wm[:, b * WC + m:b * WC + m + 1])
                    nc.sync.dma_start(out_d[b, m * 128:(m + 1) * 128, :], osb[:])

    nc.compile()
    return nc


def _get_nc():
    if "nc" not in _CACHE:
        _CACHE["nc"] = _build()
        _CACHE["consts"] = _consts()
    return _CACHE["nc"], _CACHE["consts"]


def _run(anonymized_nodes, walk_masks, trace=False):
    from concourse.bass_utils import run_bass_kernel_spmd

    nodes = np.asarray(anonymized_nodes).astype(np.int32)
    masks = np.asarray(walk_masks).astype(np.int32)

    nc, cst = _get_nc()
    in_maps = []
    for i in range(NCORES):
        m = {"nodes": nodes[i * BPC:(i + 1) * BPC],
             "masks": masks[i * BPC:(i + 1) * BPC]}
        m.update(cst)
        in_maps.append(m)
    res = run_bass_kernel_spmd(nc, in_maps, core_ids=list(range(NCORES)),
                               trace=trace)
    out = np.concatenate([res.results[i]["out"] for i in range(NCORES)], axis=0)
    return out.astype(np.float32), res


def kernel(anonymized_nodes, walk_masks, kernel):
    out, _ = _run(anonymized_nodes, walk_masks)
    return out


def run_traced(anonymized_nodes, walk_masks, kernel):
    """Like kernel() but with NTFF tracing; returns (out, BassKernelResults)."""
    return _run(anonymized_nodes, walk_masks, trace=True)


# revision 10
# speedup vs baseline: 1.1494x; 1.1494x over previous
"""Trainium2 Bass kernel for nn_CooccurrenceMatrix.

Math (per batch b):
  onehot S[(v,p), w] = [nodes[b,w,p]==v] * mask[b,w,p]        (one-hot over ids)
  A_T = blockdiag(K) @ S                                      ([(v,q), w])
  cooc_raw[w,x] = sum_{(v,q)} A_T[(v,q),w] * S[(v,q),x]
  dc[w] = #singleton-group members in walk w  (groups of size 1 get their
          self-pair K[p,p]=1 removed from the diagonal)
  out[w,x] = (cooc_raw - diag(dc)) * r[w]*r[x],  r = 1/max(lens,1)

Implementation: fully data-parallel over batches; 8 batches per NeuronCore.
All matmuls in bf16 (S, A exact in bf16; K rounded -> ~0.2% rel err).
The contraction dim (v,q)=400 is split into chunks of 120/120/120/40
(6 id-blocks of 20 positions per chunk) so PSUM accumulates 4 matmuls
per output row-chunk. r[x] is pre-folded into the cooc rhs; r[w] is a
per-partition scale on the PSUM->SBUF output copy; the singleton diagonal
correction is injected into PSUM with one small matmul per row-chunk.
"""

import numpy as np

B, W, L, V = 64, 512, 20, 20
SIGMA = 2.0
NCORES = 8
BPC = B // NCORES            # batches per core
WC = W // 128                # 4 row chunks of 128 walks
CHUNK_ROWS = (120, 120, 120, 40)   # (v,p) contraction chunk heights
CHUNK_VS = (6, 6, 6, 2)            # id-blocks per chunk
PF = BPC * WC * L            # free size of the per-core node tile (8*4*20=640)
NF = BPC * 512               # all-batch walk columns (4096)

_CACHE = {}


def _consts():
    import ml_dtypes

    bf16 = ml_dtypes.bfloat16
    pos = np.arange(L, dtype=np.float64)
    K = np.exp(-((pos[:, None] - pos[None, :]) ** 2) / (SIGMA ** 2))
    Kb = K.astype(bf16)

    bdkA = np.zeros((120, 120), dtype=bf16)       # blockdiag(K) for 6 id-blocks
    for vb in range(6):
        bdkA[vb * L:(vb + 1) * L, vb * L:(vb + 1) * L] = Kb
    bdkB = bdkA[:40, :40].copy()                  # 2-block version

    onesblk = np.zeros((120, 6), dtype=np.float32)  # [p-row, id-block] membership
    for vb in range(6):
        onesblk[vb * L:(vb + 1) * L, vb] = 1.0
    onesblk2 = onesblk[:40, :2].copy()

    E6n = np.zeros((6, 120), dtype=bf16)          # NEGATED expand block -> (block,p)
    for vb in range(6):
        E6n[vb, vb * L:(vb + 1) * L] = -1.0

    ident = np.eye(128, dtype=bf16)

    vvec1 = np.full((128, 4), -1.0, dtype=np.float32)  # compare value (id+1)
    for c in range(3):
        for r in range(120):
            vvec1[r, c] = 6 * c + r // L + 1
    for r in range(40):
        vvec1[r, 3] = 18 + r // L + 1
    # rows 40:120 of chunk 3 stay -1 (never matches) so the full-height raw
    # build zeroes them and the group-size matmul sees clean zeros.

    return dict(bdkA=bdkA, bdkB=bdkB, onesblk=onesblk, onesblk2=onesblk2,
                E6n=E6n, ident=ident, vvec1=vvec1)


def _build():
    import concourse.bacc as bacc
    import concourse.mybir as mybir
    import concourse.tile as tile

    dt = mybir.dt
    f32, bf16, i32 = dt.float32, dt.bfloat16, dt.int32
    Alu = mybir.AluOpType
    X = mybir.AxisListType.X

    nc = bacc.Bacc("TRN2", target_bir_lowering=False, debug=False,
                   enable_asserts=False, num_devices=NCORES)

    nodes_d = nc.dram_tensor("nodes", [BPC, W, L], i32, kind="ExternalInput").ap()
    masks_d = nc.dram_tensor("masks", [BPC, W, L], i32, kind="ExternalInput").ap()
    bdkA_d = nc.dram_tensor("bdkA", [120, 120], bf16, kind="ExternalInput").ap()
    bdkB_d = nc.dram_tensor("bdkB", [40, 40], bf16, kind="ExternalInput").ap()
    onesblk_d = nc.dram_tensor("onesblk", [120, 6], f32, kind="ExternalInput").ap()
    onesblk2_d = nc.dram_tensor("onesblk2", [40, 2], f32, kind="ExternalInput").ap()
    E6n_d = nc.dram_tensor("E6n", [6, 120], bf16, kind="ExternalInput").ap()
    ident_d = nc.dram_tensor("ident", [128, 128], bf16, kind="ExternalInput").ap()
    vvec1_d = nc.dram_tensor("vvec1", [128, 4], f32, kind="ExternalInput").ap()
    out_d = nc.dram_tensor("out", [BPC, W, W], f32, kind="ExternalOutput").ap()

    with tile.TileContext(nc) as tc:
        with (
            tc.tile_pool(name="const", bufs=1) as cpool,
            tc.tile_pool(name="inp", bufs=1) as ipool,
            tc.tile_pool(name="work", bufs=2) as wpool,
            tc.tile_pool(name="outp", bufs=3) as opool,
        ):
            # ---- constants to SBUF ----
            bdkA = cpool.tile([120, 120], bf16)
            nc.sync.dma_start(bdkA[:], bdkA_d)
            bdkB = cpool.tile([40, 40], bf16)
            nc.sync.dma_start(bdkB[:], bdkB_d)
            onesblk = cpool.tile([120, 6], f32)
            nc.sync.dma_start(onesblk[:], onesblk_d)
            onesblk2 = cpool.tile([40, 2], f32)
            nc.sync.dma_start(onesblk2[:], onesblk2_d)
            E6n = cpool.tile([6, 120], bf16)
            nc.sync.dma_start(E6n[:], E6n_d)
            ident = cpool.tile([128, 128], bf16)
            nc.sync.dma_start(ident[:], ident_d)
            vvec1 = cpool.tile([128, 4], f32)
            nc.sync.dma_start(vvec1[:], vvec1_d)

            # ---- all-batch input load + prep ----
            nd = ipool.tile([128, PF], i32)      # [w_lane, (b, wc, p)]
            nc.sync.dma_start(
                nd[:].rearrange("wl (b wc p) -> wl b wc p", b=BPC, wc=WC),
                nodes_d.rearrange("b (wc wl) p -> wl b wc p", wl=128))
            mk = ipool.tile([128, PF], i32)
            nc.sync.dma_start(
                mk[:].rearrange("wl (b wc p) -> wl b wc p", b=BPC, wc=WC),
                masks_d.rearrange("b (wc wl) p -> wl b wc p", wl=128))

            # ne1 = (nodes+1)*mask  (0 = masked, else id+1)
            ne1 = ipool.tile([128, PF], bf16)
            nc.vector.scalar_tensor_tensor(
                ne1[:], nd[:], 1.0, mk[:], op0=Alu.add, op1=Alu.mult)

            # lens -> r = 1/max(lens,1), per (b, wc) column
            lens = ipool.tile([128, BPC * WC], f32)
            nc.vector.reduce_sum(
                lens[:], mk[:].rearrange("wl (bw p) -> wl bw p", p=L), axis=X)
            r_wm = ipool.tile([128, BPC * WC], f32)
            nc.vector.tensor_scalar_max(lens[:], lens[:], 1.0)
            nc.vector.reciprocal(r_wm[:], lens[:])
            r_bf = ipool.tile([128, BPC * WC], bf16)
            nc.vector.tensor_copy(r_bf[:], r_wm[:])

            ne1v = ne1[:].rearrange("wl (b wc p) -> wl b wc p", b=BPC, wc=WC)

            # ---- phase 1: transposes for all batches ----
            # netall: [20 pos, (b, 512 walks)]; rx_all: [1, (b, 512 walks)]
            netall = ipool.tile([120, NF], bf16)
            rx_all = ipool.tile([1, NF], bf16)
            with (
                tc.tile_pool(name="ptr", bufs=2, space="PSUM") as ptr,
                tc.tile_pool(name="pvec", bufs=2, space="PSUM") as pvec,
            ):
                for b in range(BPC):
                    psum_tr = ptr.tile([20, 512], bf16, tag="ptr")
                    for wc in range(WC):
                        nc.tensor.transpose(
                            psum_tr[:, wc * 128:(wc + 1) * 128],
                            ne1v[:, b, wc, :], ident[:])
                    nc.scalar.copy(netall[0:20, b * 512:(b + 1) * 512],
                                   psum_tr[:])
                    psum_row = pvec.tile([1, 512], bf16, tag="prow")
                    for m in range(WC):
                        nc.tensor.transpose(
                            psum_row[:, m * 128:(m + 1) * 128],
                            r_bf[:, b * WC + m:b * WC + m + 1], ident[:])
                    nc.scalar.copy(rx_all[:, b * 512:(b + 1) * 512],
                                   psum_row[:])

            # ---- phase 2: replicate across partitions ----
            # DMA is the only engine free of the 32-aligned partition-base
            # rule, so position-block replication goes through it (5 parallel
            # copies from the same source block).
            for blk in range(1, 6):
                nc.sync.dma_start(netall[blk * 20:(blk + 1) * 20, :],
                                  netall[0:20, :])
            rrep = ipool.tile([128, NF], bf16)
            nc.gpsimd.partition_broadcast(rrep[:], rx_all[:], channels=128)

            # ---- phase 3: per-batch compute ----
            p3 = tc.tile_pool(name="pa", bufs=3, space="PSUM")
            pa = p3.__enter__()
            p4 = tc.tile_pool(name="pcooc", bufs=3, space="PSUM")
            pcooc = p4.__enter__()
            p5 = tc.tile_pool(name="psmall", bufs=1, space="PSUM")
            psmall = p5.__enter__()
            for b in range(BPC):
                net = netall[:, b * 512:(b + 1) * 512]
                rrep_b = rrep[:, b * 512:(b + 1) * 512]

                psmall_t = psmall.tile([128, 512], f32, tag="small")
                acc = wpool.tile([120, 4], f32, tag="acc")
                s_raw, s_scl = [], []
                for c in range(4):
                    R = 120 if c == 3 else CHUNK_ROWS[c]   # c3 raw build full
                    sr = wpool.tile([120, 512], bf16, tag=f"sraw{c}")
                    nc.vector.tensor_scalar(
                        sr[0:R, :], net[0:R, :], vvec1[0:R, c:c + 1], 1.0,
                        op0=Alu.is_equal, op1=Alu.mult,
                        accum_out=acc[0:R, c:c + 1])
                    ss = wpool.tile([120, 512], bf16, tag=f"sscl{c}")
                    eng = nc.vector if c < 2 else nc.gpsimd
                    eng.tensor_tensor(ss[0:CHUNK_ROWS[c], :],
                                      sr[0:CHUNK_ROWS[c], :],
                                      rrep_b[0:CHUNK_ROWS[c], :], op=Alu.mult)
                    s_raw.append(sr); s_scl.append(ss)

                # group sizes cnt_v (fp32, exact) -> singleton -> per-row -sing
                nc.tensor.matmul(psmall_t[0:6, 80:84], onesblk[:], acc[:],
                                 start=True, stop=True)
                sing = wpool.tile([6, 4], bf16, tag="sing")
                nc.vector.tensor_scalar(
                    sing[:], psmall_t[0:6, 80:84], 1.0, None, op0=Alu.is_equal)
                nc.tensor.matmul(psmall_t[0:120, 88:92], E6n[:], sing[:],
                                 start=True, stop=True)
                sv = wpool.tile([120, 4], bf16, tag="sv")
                nc.scalar.copy(sv[:], psmall_t[0:120, 88:92])

                # ---- A_T with singleton fold: (blockdiag K - diag(sing)) @ S
                a_sb = []
                for c in range(4):
                    R = CHUNK_ROWS[c]
                    bk = wpool.tile([120, 120], bf16, tag=f"bkm{c}")
                    base = bdkA[:] if R == 120 else bdkB[:]
                    nc.vector.scalar_tensor_tensor(
                        bk[0:R, 0:R], ident[0:R, 0:R], sv[0:R, c:c + 1],
                        base, op0=Alu.mult, op1=Alu.add)
                    pa_t = pa.tile([120, 512], f32, tag="pa")
                    nc.tensor.matmul(pa_t[0:R, :], bk[0:R, 0:R],
                                     s_raw[c][0:R, :], start=True, stop=True)
                    asb = wpool.tile([120, 512], bf16, tag=f"asb{c}")
                    nc.scalar.copy(asb[0:R, :], pa_t[0:R, :])
                    a_sb.append(asb)

                # ---- cooc matmuls, normalize, store ----
                for m in range(WC):
                    pc_t = pcooc.tile([128, 512], f32, tag="pcooc")
                    for c in range(4):
                        R = CHUNK_ROWS[c]
                        nc.tensor.matmul(
                            pc_t[:], a_sb[c][0:R, m * 128:(m + 1) * 128],
                            s_scl[c][0:R, :], start=(c == 0), stop=(c == 3))
                    osb = opool.tile([128, 512], f32, tag="osb")
                    nc.scalar.activation(
                        osb[:], pc_t[:],
                        mybir.ActivationFunctionType.Copy,
                        scale=r_wm[:, b * WC + m:b * WC + m + 1])
                    nc.sync.dma_start(out_d[b, m * 128:(m + 1) * 128, :], osb[:])
            p5.__exit__(None, None, None)
            p4.__exit__(None, None, None)
            p3.__exit__(None, None, None)

    nc.compile()
    return nc


def _get_nc():
    if "nc" not in _CACHE:
        _CACHE["nc"] = _build()
        _CACHE["consts"] = _consts()
    return _CACHE["nc"], _CACHE["consts"]


def _run(anonymized_nodes, walk_masks, trace=False):
    from concourse.bass_utils import run_bass_kernel_spmd

    nodes = np.asarray(anonymized_nodes).astype(np.int32)
    masks = np.asarray(walk_masks).astype(np.int32)

    nc, cst = _get_nc()
    in_maps = []
    for i in range(NCORES):
        m = {"nodes": nodes[i * BPC:(i + 1) * BPC],
             "masks": masks[i * BPC:(i + 1) * BPC]}
        m.update(cst)
        in_maps.append(m)
    res = run_bass_kernel_spmd(nc, in_maps, core_ids=list(range(NCORES)),
                               trace=trace)
    out = np.concatenate([res.results[i]["out"] for i in range(NCORES)], axis=0)
    return out.astype(np.float32), res


def kernel(anonymized_nodes, walk_masks, kernel):
    out, _ = _run(anonymized_nodes, walk_masks)
    return out


def run_traced(anonymized_nodes, walk_masks, kernel):
    """Like kernel() but with NTFF tracing; returns (out, BassKernelResults)."""
    return _run(anonymized_nodes, walk_masks, trace=True)


# revision 12
# speedup vs baseline: 1.2543x; 1.0913x over previous
"""Trainium2 Bass kernel for nn_CooccurrenceMatrix.

Math (per batch b):
  onehot S[(v,p), w] = [nodes[b,w,p]==v] * mask[b,w,p]        (one-hot over ids)
  A_T = blockdiag(K) @ S                                      ([(v,q), w])
  cooc_raw[w,x] = sum_{(v,q)} A_T[(v,q),w] * S[(v,q),x]
  dc[w] = #singleton-group members in walk w  (groups of size 1 get their
          self-pair K[p,p]=1 removed from the diagonal)
  out[w,x] = (cooc_raw - diag(dc)) * r[w]*r[x],  r = 1/max(lens,1)

Implementation: fully data-parallel over batches; 8 batches per NeuronCore.
All matmuls in bf16 (S, A exact in bf16; K rounded -> ~0.2% rel err).
The contraction dim (v,q)=400 is split into chunks of 120/120/120/40
(6 id-blocks of 20 positions per chunk) so PSUM accumulates 4 matmuls
per output row-chunk. r[x] is pre-folded into the cooc rhs; r[w] is a
per-partition scale on the PSUM->SBUF output copy; the singleton diagonal
correction is injected into PSUM with one small matmul per row-chunk.
"""

import numpy as np

B, W, L, V = 64, 512, 20, 20
SIGMA = 2.0
NCORES = 8
BPC = B // NCORES            # batches per core
WC = W // 128                # 4 row chunks of 128 walks
CHUNK_ROWS = (120, 120, 120, 40)   # (v,p) contraction chunk heights
CHUNK_VS = (6, 6, 6, 2)            # id-blocks per chunk
PF = BPC * WC * L            # free size of the per-core node tile (8*4*20=640)
NF = BPC * 512               # all-batch walk columns (4096)

_CACHE = {}


def _consts():
    import ml_dtypes

    bf16 = ml_dtypes.bfloat16
    pos = np.arange(L, dtype=np.float64)
    K = np.exp(-((pos[:, None] - pos[None, :]) ** 2) / (SIGMA ** 2))
    Kb = K.astype(bf16)

    bdkA = np.zeros((120, 120), dtype=bf16)       # blockdiag(K) for 6 id-blocks
    for vb in range(6):
        bdkA[vb * L:(vb + 1) * L, vb * L:(vb + 1) * L] = Kb
    bdkB = bdkA[:40, :40].copy()                  # 2-block version

    onesblk = np.zeros((120, 6), dtype=np.float32)  # [p-row, id-block] membership
    for vb in range(6):
        onesblk[vb * L:(vb + 1) * L, vb] = 1.0
    onesblk2 = onesblk[:40, :2].copy()

    E6n = np.zeros((6, 120), dtype=bf16)          # NEGATED expand block -> (block,p)
    for vb in range(6):
        E6n[vb, vb * L:(vb + 1) * L] = -1.0

    ident = np.eye(128, dtype=bf16)

    vvec1 = np.full((128, 4), -1.0, dtype=np.float32)  # compare value (id+1)
    for c in range(3):
        for r in range(120):
            vvec1[r, c] = 6 * c + r // L + 1
    for r in range(40):
        vvec1[r, 3] = 18 + r // L + 1
    # rows 40:120 of chunk 3 stay -1 (never matches) so the full-height raw
    # build zeroes them and the group-size matmul sees clean zeros.

    return dict(bdkA=bdkA, bdkB=bdkB, onesblk=onesblk, onesblk2=onesblk2,
                E6n=E6n, ident=ident, vvec1=vvec1)


def _build():
    import concourse.bacc as bacc
    import concourse.mybir as mybir
    import concourse.tile as tile

    dt = mybir.dt
    f32, bf16, i32 = dt.float32, dt.bfloat16, dt.int32
    Alu = mybir.AluOpType
    X = mybir.AxisListType.X

    nc = bacc.Bacc("TRN2", target_bir_lowering=False, debug=False,
                   enable_asserts=False, num_devices=NCORES)

    nodes_d = nc.dram_tensor("nodes", [BPC, W, L], i32, kind="ExternalInput").ap()
    masks_d = nc.dram_tensor("masks", [BPC, W, L], i32, kind="ExternalInput").ap()
    bdkA_d = nc.dram_tensor("bdkA", [120, 120], bf16, kind="ExternalInput").ap()
    bdkB_d = nc.dram_tensor("bdkB", [40, 40], bf16, kind="ExternalInput").ap()
    onesblk_d = nc.dram_tensor("onesblk", [120, 6], f32, kind="ExternalInput").ap()
    onesblk2_d = nc.dram_tensor("onesblk2", [40, 2], f32, kind="ExternalInput").ap()
    E6n_d = nc.dram_tensor("E6n", [6, 120], bf16, kind="ExternalInput").ap()
    ident_d = nc.dram_tensor("ident", [128, 128], bf16, kind="ExternalInput").ap()
    vvec1_d = nc.dram_tensor("vvec1", [128, 4], f32, kind="ExternalInput").ap()
    out_d = nc.dram_tensor("out", [BPC, W, W], f32, kind="ExternalOutput").ap()

    with tile.TileContext(nc) as tc:
        with (
            tc.tile_pool(name="const", bufs=1) as cpool,
            tc.tile_pool(name="inp", bufs=1) as ipool,
            tc.tile_pool(name="work", bufs=3) as wpool,
            tc.tile_pool(name="outp", bufs=3) as opool,
        ):
            # ---- constants to SBUF ----
            bdkA = cpool.tile([120, 120], bf16)
            nc.sync.dma_start(bdkA[:], bdkA_d)
            bdkB = cpool.tile([40, 40], bf16)
            nc.sync.dma_start(bdkB[:], bdkB_d)
            onesblk = cpool.tile([120, 6], f32)
            nc.sync.dma_start(onesblk[:], onesblk_d)
            onesblk2 = cpool.tile([40, 2], f32)
            nc.sync.dma_start(onesblk2[:], onesblk2_d)
            E6n = cpool.tile([6, 120], bf16)
            nc.sync.dma_start(E6n[:], E6n_d)
            ident = cpool.tile([128, 128], bf16)
            nc.sync.dma_start(ident[:], ident_d)
            vvec1 = cpool.tile([128, 4], f32)
            nc.sync.dma_start(vvec1[:], vvec1_d)

            # ---- all-batch input load + prep ----
            nd = ipool.tile([128, PF], i32)      # [w_lane, (b, wc, p)]
            nc.sync.dma_start(
                nd[:].rearrange("wl (b wc p) -> wl b wc p", b=BPC, wc=WC),
                nodes_d.rearrange("b (wc wl) p -> wl b wc p", wl=128))
            mk = ipool.tile([128, PF], i32)
            nc.sync.dma_start(
                mk[:].rearrange("wl (b wc p) -> wl b wc p", b=BPC, wc=WC),
                masks_d.rearrange("b (wc wl) p -> wl b wc p", wl=128))

            # ne1 = (nodes+1)*mask  (0 = masked, else id+1)
            ne1 = ipool.tile([128, PF], bf16)
            nc.vector.scalar_tensor_tensor(
                ne1[:], nd[:], 1.0, mk[:], op0=Alu.add, op1=Alu.mult)

            # lens -> r = 1/max(lens,1), per (b, wc) column
            lens = ipool.tile([128, BPC * WC], f32)
            nc.vector.reduce_sum(
                lens[:], mk[:].rearrange("wl (bw p) -> wl bw p", p=L), axis=X)
            r_wm = ipool.tile([128, BPC * WC], f32)
            nc.vector.tensor_scalar_max(lens[:], lens[:], 1.0)
            nc.vector.reciprocal(r_wm[:], lens[:])
            r_bf = ipool.tile([128, BPC * WC], bf16)
            nc.vector.tensor_copy(r_bf[:], r_wm[:])

            ne1v = ne1[:].rearrange("wl (b wc p) -> wl b wc p", b=BPC, wc=WC)

            # netall: [20 pos, (b, 512 walks)]; rx_all: [1, (b, 512 walks)]
            netall = ipool.tile([120, NF], bf16)
            rx_all = ipool.tile([1, NF], bf16)
            rrep = ipool.tile([128, NF], bf16)

            with (
                tc.tile_pool(name="ptr", bufs=2, space="PSUM") as ptr,
                tc.tile_pool(name="pa", bufs=2, space="PSUM") as pa,
                tc.tile_pool(name="pcooc", bufs=3, space="PSUM") as pcooc,
                tc.tile_pool(name="psmall", bufs=1, space="PSUM") as psmall,
            ):
                # ---- phase 1: transposes + replication, two batches at a
                # time so phase-3 work can start while later pairs transpose.
                for b0 in range(0, BPC, 2):
                    for b in (b0, b0 + 1):
                        psum_row = ptr.tile([20, 512], bf16, tag="ptr")
                        for m in range(WC):
                            nc.tensor.transpose(
                                psum_row[0:1, m * 128:(m + 1) * 128],
                                r_bf[:, b * WC + m:b * WC + m + 1], ident[:])
                        nc.scalar.copy(rx_all[:, b * 512:(b + 1) * 512],
                                       psum_row[0:1, :])
                        psum_tr = ptr.tile([20, 512], bf16, tag="ptr")
                        for wc in range(WC):
                            nc.tensor.transpose(
                                psum_tr[:, wc * 128:(wc + 1) * 128],
                                ne1v[:, b, wc, :], ident[:])
                        nc.scalar.copy(netall[0:20, b * 512:(b + 1) * 512],
                                       psum_tr[:])
                    cols = slice(b0 * 512, (b0 + 2) * 512)
                    nc.gpsimd.partition_broadcast(
                        rrep[:, cols], rx_all[:, cols], channels=128)
                    for blk in range(1, 6):
                        nc.sync.dma_start(
                            netall[blk * 20:(blk + 1) * 20, cols],
                            netall[0:20, cols])

                # ---- phase 3: per-batch compute ----
                for b in range(BPC):
                    net = netall[:, b * 512:(b + 1) * 512]
                    rrep_b = rrep[:, b * 512:(b + 1) * 512]

                    psmall_t = psmall.tile([128, 512], f32, tag="small")
                    acc = wpool.tile([120, 4], f32, tag="acc")
                    s_raw, s_scl = [], []
                    for c in range(4):
                        R = 120 if c == 3 else CHUNK_ROWS[c]  # c3 raw full
                        sr = wpool.tile([120, 512], bf16, tag=f"sraw{c}")
                        nc.vector.tensor_scalar(
                            sr[0:R, :], net[0:R, :], vvec1[0:R, c:c + 1], 1.0,
                            op0=Alu.is_equal, op1=Alu.mult,
                            accum_out=acc[0:R, c:c + 1])
                        ss = wpool.tile([120, 512], bf16, tag=f"sscl{c}")
                        nc.vector.tensor_tensor(
                            ss[0:CHUNK_ROWS[c], :], sr[0:CHUNK_ROWS[c], :],
                            rrep_b[0:CHUNK_ROWS[c], :], op=Alu.mult)
                        s_raw.append(sr); s_scl.append(ss)

                    # group sizes cnt_v -> singleton -> per-(id,p) -sing rows
                    nc.tensor.matmul(psmall_t[0:6, 80:84], onesblk[:], acc[:],
                                     start=True, stop=True)
                    sing = wpool.tile([6, 4], bf16, tag="sing")
                    nc.vector.tensor_scalar(
                        sing[:], psmall_t[0:6, 80:84], 1.0, None,
                        op0=Alu.is_equal)
                    nc.tensor.matmul(psmall_t[0:120, 88:92], E6n[:], sing[:],
                                     start=True, stop=True)

                    # ---- A_T with fold: (blockdiag K - diag(sing)) @ S ----
                    a_sb = []
                    for c in range(4):
                        R = CHUNK_ROWS[c]
                        bk = wpool.tile([120, 120], bf16, tag=f"bkm{c}")
                        base = bdkA[:] if R == 120 else bdkB[:]
                        nc.vector.scalar_tensor_tensor(
                            bk[0:R, 0:R], ident[0:R, 0:R],
                            psmall_t[0:R, 88 + c:89 + c],
                            base, op0=Alu.mult, op1=Alu.add)
                        pa_t = pa.tile([120, 512], f32, tag="pa")
                        nc.tensor.matmul(pa_t[0:R, :], bk[0:R, 0:R],
                                         s_raw[c][0:R, :],
                                         start=True, stop=True)
                        asb = wpool.tile([120, 512], bf16, tag=f"asb{c}")
                        nc.scalar.copy(asb[0:R, :], pa_t[0:R, :])
                        a_sb.append(asb)

                    # ---- cooc matmuls, normalize, store ----
                    for m in range(WC):
                        pc_t = pcooc.tile([128, 512], f32, tag="pcooc")
                        for c in range(4):
                            R = CHUNK_ROWS[c]
                            nc.tensor.matmul(
                                pc_t[:], a_sb[c][0:R, m * 128:(m + 1) * 128],
                                s_scl[c][0:R, :], start=(c == 0), stop=(c == 3))
                        osb = opool.tile([128, 512], f32, tag="osb")
                        nc.scalar.activation(
                            osb[:], pc_t[:],
                            mybir.ActivationFunctionType.Copy,
                            scale=r_wm[:, b * WC + m:b * WC + m + 1])
                        nc.sync.dma_start(out_d[b, m * 128:(m + 1) * 128, :],
                                          osb[:])

    nc.compile()
    return nc


def _get_nc():
    if "nc" not in _CACHE:
        _CACHE["nc"] = _build()
        _CACHE["consts"] = _consts()
    return _CACHE["nc"], _CACHE["consts"]


def _run(anonymized_nodes, walk_masks, trace=False):
    from concourse.bass_utils import run_bass_kernel_spmd

    nodes = np.asarray(anonymized_nodes).astype(np.int32)
    masks = np.asarray(walk_masks).astype(np.int32)

    nc, cst = _get_nc()
    in_maps = []
    for i in range(NCORES):
        m = {"nodes": nodes[i * BPC:(i + 1) * BPC],
             "masks": masks[i * BPC:(i + 1) * BPC]}
        m.update(cst)
        in_maps.append(m)
    res = run_bass_kernel_spmd(nc, in_maps, core_ids=list(range(NCORES)),
                               trace=trace)
    out = np.concatenate([res.results[i]["out"] for i in range(NCORES)], axis=0)
    return out.astype(np.float32), res


def kernel(anonymized_nodes, walk_masks, kernel):
    out, _ = _run(anonymized_nodes, walk_masks)
    return out


def run_traced(anonymized_nodes, walk_masks, kernel):
    """Like kernel() but with NTFF tracing; returns (out, BassKernelResults)."""
    return _run(anonymized_nodes, walk_masks, trace=True)


# revision 15
# speedup vs baseline: 1.4095x; 1.1237x over previous
"""Trainium2 Bass kernel for nn_CooccurrenceMatrix.

Math (per batch b):
  onehot S[(v,p), w] = [nodes[b,w,p]==v] * mask[b,w,p]        (one-hot over ids)
  A_T = blockdiag(K) @ S                                      ([(v,q), w])
  cooc_raw[w,x] = sum_{(v,q)} A_T[(v,q),w] * S[(v,q),x]
  dc[w] = #singleton-group members in walk w  (groups of size 1 get their
          self-pair K[p,p]=1 removed from the diagonal)
  out[w,x] = (cooc_raw - diag(dc)) * r[w]*r[x],  r = 1/max(lens,1)

Implementation: fully data-parallel over batches; 8 batches per NeuronCore.
All matmuls in bf16 (S, A exact in bf16; K rounded -> ~0.2% rel err).
The contraction dim (v,q)=400 is split into chunks of 120/120/120/40
(6 id-blocks of 20 positions per chunk) so PSUM accumulates 4 matmuls
per output row-chunk. r[x] is pre-folded into the cooc rhs; r[w] is a
per-partition scale on the PSUM->SBUF output copy; the singleton diagonal
correction is injected into PSUM with one small matmul per row-chunk.
"""

import numpy as np

B, W, L, V = 64, 512, 20, 20
SIGMA = 2.0
NCORES = 8
BPC = B // NCORES            # batches per core
WC = W // 128                # 4 row chunks of 128 walks
CHUNK_ROWS = (120, 120, 120, 40)   # (v,p) contraction chunk heights
CHUNK_VS = (6, 6, 6, 2)            # id-blocks per chunk
PF = BPC * WC * L            # free size of the per-core node tile (8*4*20=640)
NF = BPC * 512               # all-batch walk columns (4096)

_CACHE = {}


def _consts():
    import ml_dtypes

    bf16 = ml_dtypes.bfloat16
    pos = np.arange(L, dtype=np.float64)
    K = np.exp(-((pos[:, None] - pos[None, :]) ** 2) / (SIGMA ** 2))
    Kb = K.astype(bf16)

    bdkA = np.zeros((120, 120), dtype=bf16)       # blockdiag(K) for 6 id-blocks
    for vb in range(6):
        bdkA[vb * L:(vb + 1) * L, vb * L:(vb + 1) * L] = Kb
    bdkB = bdkA[:40, :40].copy()                  # 2-block version

    onesblk = np.zeros((120, 6), dtype=np.float32)  # [p-row, id-block] membership
    for vb in range(6):
        onesblk[vb * L:(vb + 1) * L, vb] = 1.0
    onesblk2 = onesblk[:40, :2].copy()

    E6n = np.zeros((6, 120), dtype=bf16)          # NEGATED expand block -> (block,p)
    for vb in range(6):
        E6n[vb, vb * L:(vb + 1) * L] = -1.0

    ident = np.eye(128, dtype=bf16)

    vvec1 = np.full((128, 4), -1.0, dtype=np.float32)  # compare value (id+1)
    for c in range(3):
        for r in range(120):
            vvec1[r, c] = 6 * c + r // L + 1
    for r in range(40):
        vvec1[r, 3] = 18 + r // L + 1
    # rows 40:120 of chunk 3 stay -1 (never matches) so the full-height raw
    # build zeroes them and the group-size matmul sees clean zeros.

    return dict(bdkA=bdkA, bdkB=bdkB, onesblk=onesblk, onesblk2=onesblk2,
                E6n=E6n, ident=ident, vvec1=vvec1)


def _build():
    import concourse.bacc as bacc
    import concourse.mybir as mybir
    import concourse.tile as tile

    dt = mybir.dt
    f32, bf16, i32 = dt.float32, dt.bfloat16, dt.int32
    Alu = mybir.AluOpType
    X = mybir.AxisListType.X

    nc = bacc.Bacc("TRN2", target_bir_lowering=False, debug=False,
                   enable_asserts=False, num_devices=NCORES)

    # host pre-strides to [w_lane, b, wc, p] so this DMA is fully contiguous
    nodes_d = nc.dram_tensor("nodes", [128, PF], i32, kind="ExternalInput").ap()
    masks_d = nc.dram_tensor("masks", [128, PF], i32, kind="ExternalInput").ap()
    bdkA_d = nc.dram_tensor("bdkA", [120, 120], bf16, kind="ExternalInput").ap()
    bdkB_d = nc.dram_tensor("bdkB", [40, 40], bf16, kind="ExternalInput").ap()
    onesblk_d = nc.dram_tensor("onesblk", [120, 6], f32, kind="ExternalInput").ap()
    onesblk2_d = nc.dram_tensor("onesblk2", [40, 2], f32, kind="ExternalInput").ap()
    E6n_d = nc.dram_tensor("E6n", [6, 120], bf16, kind="ExternalInput").ap()
    ident_d = nc.dram_tensor("ident", [128, 128], bf16, kind="ExternalInput").ap()
    vvec1_d = nc.dram_tensor("vvec1", [128, 4], f32, kind="ExternalInput").ap()
    out_d = nc.dram_tensor("out", [BPC, W, W], f32, kind="ExternalOutput").ap()

    with tile.TileContext(nc) as tc:
        with (
            tc.tile_pool(name="const", bufs=1) as cpool,
            tc.tile_pool(name="inp", bufs=1) as ipool,
            tc.tile_pool(name="work", bufs=3) as wpool,
            tc.tile_pool(name="outp", bufs=3) as opool,
        ):
            # ---- all-batch input load (contiguous, first in the queue) ----
            nd = ipool.tile([128, PF], i32)      # [w_lane, (b, wc, p)]
            nc.sync.dma_start(nd[:], nodes_d)
            mk = ipool.tile([128, PF], i32)
            nc.scalar.dma_start(mk[:], masks_d)

            # ---- constants to SBUF (SWDGE queue, parallel to inputs) ----
            bdkA = cpool.tile([120, 120], bf16)
            nc.gpsimd.dma_start(bdkA[:], bdkA_d)
            bdkB = cpool.tile([40, 40], bf16)
            nc.gpsimd.dma_start(bdkB[:], bdkB_d)
            onesblk = cpool.tile([120, 6], f32)
            nc.gpsimd.dma_start(onesblk[:], onesblk_d)
            onesblk2 = cpool.tile([40, 2], f32)
            nc.gpsimd.dma_start(onesblk2[:], onesblk2_d)
            E6n = cpool.tile([6, 120], bf16)
            nc.gpsimd.dma_start(E6n[:], E6n_d)
            ident = cpool.tile([128, 128], bf16)
            nc.gpsimd.dma_start(ident[:], ident_d)
            vvec1 = cpool.tile([128, 4], f32)
            nc.gpsimd.dma_start(vvec1[:], vvec1_d)

            # ne1 = (nodes+1)*mask  (0 = masked, else id+1)
            ne1 = ipool.tile([128, PF], bf16)
            nc.vector.scalar_tensor_tensor(
                ne1[:], nd[:], 1.0, mk[:], op0=Alu.add, op1=Alu.mult)

            # lens -> r = 1/max(lens,1), per (b, wc) column
            lens = ipool.tile([128, BPC * WC], f32)
            nc.vector.reduce_sum(
                lens[:], mk[:].rearrange("wl (bw p) -> wl bw p", p=L), axis=X)
            r_wm = ipool.tile([128, BPC * WC], f32)
            nc.vector.tensor_scalar_max(lens[:], lens[:], 1.0)
            nc.vector.reciprocal(r_wm[:], lens[:])
            r_bf = ipool.tile([128, BPC * WC], bf16)
            nc.vector.tensor_copy(r_bf[:], r_wm[:])

            ne1v = ne1[:].rearrange("wl (b wc p) -> wl b wc p", b=BPC, wc=WC)

            # netall: [20 pos, (b, 512 walks)]; rx_all: [1, (b, 512 walks)]
            netall = ipool.tile([120, NF], bf16)
            rx_all = ipool.tile([1, NF], bf16)
            rrep = ipool.tile([128, NF], bf16)

            with (
                tc.tile_pool(name="ptr", bufs=2, space="PSUM") as ptr,
                tc.tile_pool(name="pa", bufs=2, space="PSUM") as pa,
                tc.tile_pool(name="pcooc", bufs=3, space="PSUM") as pcooc,
                tc.tile_pool(name="psmall", bufs=1, space="PSUM") as psmall,
            ):
                # ---- phase 1: transposes + replication, two batches at a
                # time so phase-3 work can start while later pairs transpose.
                for b0 in range(0, BPC, 2):
                    for b in (b0, b0 + 1):
                        psum_row = ptr.tile([20, 512], bf16, tag="ptr")
                        for m in range(WC):
                            nc.tensor.transpose(
                                psum_row[0:1, m * 128:(m + 1) * 128],
                                r_bf[:, b * WC + m:b * WC + m + 1], ident[:])
                        nc.scalar.copy(rx_all[:, b * 512:(b + 1) * 512],
                                       psum_row[0:1, :])
                        psum_tr = ptr.tile([20, 512], bf16, tag="ptr")
                        for wc in range(WC):
                            nc.tensor.transpose(
                                psum_tr[:, wc * 128:(wc + 1) * 128],
                                ne1v[:, b, wc, :], ident[:])
                        nc.scalar.copy(netall[0:20, b * 512:(b + 1) * 512],
                                       psum_tr[:])
                    cols = slice(b0 * 512, (b0 + 2) * 512)
                    nc.gpsimd.partition_broadcast(
                        rrep[:, cols], rx_all[:, cols], channels=128)
                    for blk in range(1, 6):
                        nc.sync.dma_start(
                            netall[blk * 20:(blk + 1) * 20, cols],
                            netall[0:20, cols])

                # ---- phase 3: per-batch compute ----
                for b in range(BPC):
                    net = netall[:, b * 512:(b + 1) * 512]
                    rrep_b = rrep[:, b * 512:(b + 1) * 512]

                    psmall_t = psmall.tile([128, 512], f32, tag="small")
                    acc = wpool.tile([120, 4], f32, tag="acc")
                    s_raw, s_scl = [], []
                    for c in range(4):
                        R = 120 if c == 3 else CHUNK_ROWS[c]  # c3 raw full
                        sr = wpool.tile([120, 512], bf16, tag=f"sraw{c}")
                        nc.vector.tensor_scalar(
                            sr[0:R, :], net[0:R, :], vvec1[0:R, c:c + 1], 1.0,
                            op0=Alu.is_equal, op1=Alu.mult,
                            accum_out=acc[0:R, c:c + 1])
                        ss = wpool.tile([120, 512], bf16, tag=f"sscl{c}")
                        nc.vector.tensor_tensor(
                            ss[0:CHUNK_ROWS[c], :], sr[0:CHUNK_ROWS[c], :],
                            rrep_b[0:CHUNK_ROWS[c], :], op=Alu.mult)
                        s_raw.append(sr); s_scl.append(ss)

                    # group sizes cnt_v -> singleton -> per-(id,p) -sing rows
                    nc.tensor.matmul(psmall_t[0:6, 80:84], onesblk[:], acc[:],
                                     start=True, stop=True)
                    sing = wpool.tile([6, 4], bf16, tag="sing")
                    nc.vector.tensor_scalar(
                        sing[:], psmall_t[0:6, 80:84], 1.0, None,
                        op0=Alu.is_equal)
                    nc.tensor.matmul(psmall_t[0:120, 88:92], E6n[:], sing[:],
                                     start=True, stop=True)

                    # ---- A_T with fold: (blockdiag K - diag(sing)) @ S ----
                    a_sb = []
                    for c in range(4):
                        R = CHUNK_ROWS[c]
                        bk = wpool.tile([120, 120], bf16, tag=f"bkm{c}")
                        base = bdkA[:] if R == 120 else bdkB[:]
                        nc.vector.scalar_tensor_tensor(
                            bk[0:R, 0:R], ident[0:R, 0:R],
                            psmall_t[0:R, 88 + c:89 + c],
                            base, op0=Alu.mult, op1=Alu.add)
                        pa_t = pa.tile([120, 512], f32, tag="pa")
                        nc.tensor.matmul(pa_t[0:R, :], bk[0:R, 0:R],
                                         s_raw[c][0:R, :],
                                         start=True, stop=True)
                        asb = wpool.tile([120, 512], bf16, tag=f"asb{c}")
                        nc.scalar.copy(asb[0:R, :], pa_t[0:R, :])
                        a_sb.append(asb)

                    # ---- cooc matmuls, normalize, store ----
                    for m in range(WC):
                        pc_t = pcooc.tile([128, 512], f32, tag="pcooc")
                        for c in range(4):
                            R = CHUNK_ROWS[c]
                            nc.tensor.matmul(
                                pc_t[:], a_sb[c][0:R, m * 128:(m + 1) * 128],
                                s_scl[c][0:R, :], start=(c == 0), stop=(c == 3))
                        osb = opool.tile([128, 512], f32, tag="osb")
                        nc.scalar.activation(
                            osb[:], pc_t[:],
                            mybir.ActivationFunctionType.Copy,
                            scale=r_wm[:, b * WC + m:b * WC + m + 1])
                        nc.sync.dma_start(out_d[b, m * 128:(m + 1) * 128, :],
                                          osb[:])

    nc.compile()
    return nc


def _get_nc():
    if "nc" not in _CACHE:
        _CACHE["nc"] = _build()
        _CACHE["consts"] = _consts()
    return _CACHE["nc"], _CACHE["consts"]


def _run(anonymized_nodes, walk_masks, trace=False):
    from concourse.bass_utils import run_bass_kernel_spmd

    nodes = np.asarray(anonymized_nodes).astype(np.int32)
    masks = np.asarray(walk_masks).astype(np.int32)

    def restride(x):
        # [BPC, 512, 20] -> [w_lane 128, (b, wc, p)] contiguous
        return np.ascontiguousarray(
            x.reshape(BPC, WC, 128, L).transpose(2, 0, 1, 3)
        ).reshape(128, PF)

    nc, cst = _get_nc()
    in_maps = []
    for i in range(NCORES):
        m = {"nodes": restride(nodes[i * BPC:(i + 1) * BPC]),
             "masks": restride(masks[i * BPC:(i + 1) * BPC])}
        m.update(cst)
        in_maps.append(m)
    res = run_bass_kernel_spmd(nc, in_maps, core_ids=list(range(NCORES)),
                               trace=trace)
    out = np.concatenate([res.results[i]["out"] for i in range(NCORES)], axis=0)
    return out.astype(np.float32), res


def kernel(anonymized_nodes, walk_masks, kernel):
    out, _ = _run(anonymized_nodes, walk_masks)
    return out


def run_traced(anonymized_nodes, walk_masks, kernel):
    """Like kernel() but with NTFF tracing; returns (out, BassKernelResults)."""
    return _run(anonymized_nodes, walk_masks, trace=True)
